# revision 3
# baseline (speedup 1.0000x reference)
"""CLAPP layer step on 8 NeuronCores (Bass/Tile).

Sharding: W / dW / mem / spk_trace / prediction / negative_spk_trace are
row-sharded over the hidden dim (1024 rows per core); inp / inp_trace are
replicated. The two scalar losses are combined with one on-device AllReduce.

Per-core layout: local hidden row h = p*8 + b  (p = SBUF partition 0..127,
b = block 0..7), so [1024] vectors map to [128, 8] tiles with 32-byte
per-partition DMA descriptors and W/dW 128-row blocks are row-strided by 8.
"""

import numpy as np
from contextlib import ExitStack

NCORES = 8
NH = 8192
NIN = 8192
SH = NH // NCORES          # 1024 rows per core
P = 128
B = SH // P                # 8 blocks
KCH = 4096                 # k-chunk width
NKCH = NIN // KCH          # 2 chunks
BETA = 0.9
THR = 1.0
PI = float(np.pi)

_cache = {}


def _build_nc():
    from concourse import bacc, tile, mybir

    F32 = mybir.dt.float32
    Alu = mybir.AluOpType
    Act = mybir.ActivationFunctionType

    nc = bacc.Bacc(
        "TRN2",
        target_bir_lowering=False,
        debug=False,
        enable_asserts=False,
        num_devices=NCORES,
    )

    W = nc.dram_tensor("W", [SH, NIN], F32, kind="ExternalInput")
    inp = nc.dram_tensor("inp", [NIN], F32, kind="ExternalInput")
    mem = nc.dram_tensor("mem", [SH], F32, kind="ExternalInput")
    sptr = nc.dram_tensor("sptr", [SH], F32, kind="ExternalInput")
    inptr = nc.dram_tensor("inptr", [NIN], F32, kind="ExternalInput")
    pred = nc.dram_tensor("pred", [SH], F32, kind="ExternalInput")
    negt = nc.dram_tensor("negt", [SH], F32, kind="ExternalInput")

    spk_o = nc.dram_tensor("spk_o", [SH], F32, kind="ExternalOutput")
    st_o = nc.dram_tensor("st_o", [SH], F32, kind="ExternalOutput")
    loss_o = nc.dram_tensor("loss_o", [1], F32, kind="ExternalOutput")
    dW_o = nc.dram_tensor("dW_o", [SH, NIN], F32, kind="ExternalOutput")

    W_r = W.ap().rearrange("(p b) k -> p b k", b=B)
    dW_r = dW_o.ap().rearrange("(p b) k -> p b k", b=B)

    def shard_2d(t):
        return t.ap().rearrange("(p b) -> p b", b=B)

    with tile.TileContext(nc) as tc, ExitStack() as ctx:
        singles = ctx.enter_context(tc.tile_pool(name="singles", bufs=1))
        wpool = ctx.enter_context(tc.tile_pool(name="wpool", bufs=3))
        dwpool = ctx.enter_context(tc.tile_pool(name="dwpool", bufs=3))
        psum = ctx.enter_context(tc.tile_pool(name="psum", bufs=1, space="PSUM"))
        dram = ctx.enter_context(tc.tile_pool(name="dram", bufs=1, space="DRAM"))

        # --- replicated row vectors, broadcast across partitions ---
        inp_bc = singles.tile([P, NIN], F32)   # becomes it_bc in place later
        tr_bc = singles.tile([P, NIN], F32)
        nc.sync.dma_start(
            inp_bc[:], inp.ap().rearrange("(a k) -> a k", a=1).broadcast_to([P, NIN])
        )
        nc.sync.dma_start(
            tr_bc[:], inptr.ap().rearrange("(a k) -> a k", a=1).broadcast_to([P, NIN])
        )

        # --- sharded small vectors as [128, 8] ---
        mem_t = singles.tile([P, B], F32)
        sptr_t = singles.tile([P, B], F32)
        pred_t = singles.tile([P, B], F32)
        negt_t = singles.tile([P, B], F32)
        nc.sync.dma_start(mem_t[:], shard_2d(mem))
        nc.sync.dma_start(sptr_t[:], shard_2d(sptr))
        nc.sync.dma_start(pred_t[:], shard_2d(pred))
        nc.sync.dma_start(negt_t[:], shard_2d(negt))

        ones = singles.tile([P, 1], F32)
        nc.vector.memset(ones[:], 1.0)

        # --- matvec: cur[h] = sum_k W[h, k] * inp[k] ---
        acc = singles.tile([P, B, NKCH], F32)
        scr = singles.tile([P, KCH], F32)
        for b in range(B):
            for j in range(NKCH):
                w_t = wpool.tile([P, KCH], F32, tag="w")
                nc.sync.dma_start(w_t[:], W_r[:, b, j * KCH:(j + 1) * KCH])
                nc.vector.scalar_tensor_tensor(
                    out=scr[:],
                    in0=w_t[:],
                    scalar=1.0,
                    op0=Alu.bypass,
                    in1=inp_bc[:, j * KCH:(j + 1) * KCH],
                    op1=Alu.mult,
                    accum_out=acc[:, b, j:j + 1],
                )
        cur_t = singles.tile([P, B], F32)
        nc.vector.tensor_reduce(
            out=cur_t[:], in_=acc[:], axis=mybir.AxisListType.X, op=Alu.add
        )

        # --- LIF neuron step ---
        rst_t = singles.tile([P, B], F32)
        nc.vector.tensor_scalar(
            out=rst_t[:], in0=mem_t[:], scalar1=THR, scalar2=None, op0=Alu.is_gt
        )
        mn0_t = singles.tile([P, B], F32)
        nc.vector.scalar_tensor_tensor(
            out=mn0_t[:], in0=mem_t[:], scalar=BETA, op0=Alu.mult,
            in1=cur_t[:], op1=Alu.add,
        )
        mnew_t = singles.tile([P, B], F32)
        nc.vector.tensor_tensor(
            out=mnew_t[:], in0=mn0_t[:], in1=rst_t[:], op=Alu.subtract
        )
        spk_t = singles.tile([P, B], F32)
        nc.vector.tensor_scalar(
            out=spk_t[:], in0=mnew_t[:], scalar1=THR, scalar2=None, op0=Alu.is_gt
        )
        st_t = singles.tile([P, B], F32)
        nc.vector.scalar_tensor_tensor(
            out=st_t[:], in0=spk_t[:], scalar=0.1, op0=Alu.mult,
            in1=sptr_t[:], op1=Alu.add,
        )
        nc.sync.dma_start(shard_2d(spk_o), spk_t[:])
        nc.sync.dma_start(shard_2d(st_o), st_t[:])

        # --- partial losses: [sum (pred-st)^2, sum (negt-pred)^2] ---
        pms_t = singles.tile([P, B], F32)
        nc.vector.tensor_tensor(
            out=pms_t[:], in0=pred_t[:], in1=st_t[:], op=Alu.subtract
        )
        nmp_t = singles.tile([P, B], F32)
        nc.vector.tensor_tensor(
            out=nmp_t[:], in0=negt_t[:], in1=pred_t[:], op=Alu.subtract
        )
        plos = singles.tile([P, 2], F32)
        sq_scr = singles.tile([P, B], F32)
        nc.vector.scalar_tensor_tensor(
            out=sq_scr[:], in0=pms_t[:], scalar=1.0, op0=Alu.bypass,
            in1=pms_t[:], op1=Alu.mult, accum_out=plos[:, 0:1],
        )
        sq_scr2 = singles.tile([P, B], F32)
        nc.vector.scalar_tensor_tensor(
            out=sq_scr2[:], in0=nmp_t[:], scalar=1.0, op0=Alu.bypass,
            in1=nmp_t[:], op1=Alu.mult, accum_out=plos[:, 1:2],
        )
        lsum_ps = psum.tile([1, 2], F32)
        nc.tensor.matmul(lsum_ps[:], ones[:], plos[:])
        lsum_sb = singles.tile([1, 2], F32)
        nc.vector.tensor_copy(lsum_sb[:], lsum_ps[:])

        # --- AllReduce the two partial sums across the 8 cores ---
        cc_in = dram.tile([1, 2], F32)
        cc_out = dram.tile([1, 2], F32)
        nc.gpsimd.dma_start(cc_in[:], lsum_sb[:])
        nc.gpsimd.collective_compute(
            "AllReduce",
            Alu.add,
            replica_groups=[list(range(NCORES))],
            ins=[cc_in.opt()],
            outs=[cc_out.opt()],
        )
        ar_bc = singles.tile([P, 2], F32)
        nc.sync.dma_start(ar_bc[:], cc_out[:].broadcast_to([P, 2]))

        # --- loss output: (L - S) / 2  (loss_c = -S) ---
        ld = singles.tile([1, 1], F32)
        nc.vector.tensor_tensor(
            out=ld[:], in0=ar_bc[0:1, 0:1], in1=ar_bc[0:1, 1:2], op=Alu.subtract
        )
        nc.vector.tensor_scalar(
            out=ld[:], in0=ld[:], scalar1=0.5, scalar2=None, op0=Alu.mult
        )
        nc.sync.dma_start(loss_o.ap().rearrange("(a k) -> a k", a=1), ld[:])

        # --- flags and combined gradient row vector g ---
        a_t = singles.tile([P, 1], F32)
        nc.vector.tensor_scalar(
            out=a_t[:], in0=ar_bc[:, 0:1], scalar1=20.0, scalar2=None, op0=Alu.is_gt
        )
        b_t = singles.tile([P, 1], F32)
        nc.vector.tensor_scalar(
            out=b_t[:], in0=ar_bc[:, 1:2], scalar1=100.0, scalar2=None, op0=Alu.is_lt
        )
        # -surr = -1 / (pi * (1 + (pi*(mem_new-1))^2))
        negpi_t = singles.tile([P, 1], F32)
        nc.vector.memset(negpi_t[:], -PI)
        ssq_t = singles.tile([P, B], F32)
        nc.scalar.activation(
            out=ssq_t[:], in_=mnew_t[:], func=Act.Square, bias=negpi_t[:, 0:1],
            scale=PI,
        )
        rn_t = singles.tile([P, B], F32)
        nc.vector.tensor_scalar(
            out=rn_t[:], in0=ssq_t[:], scalar1=-PI, scalar2=-PI,
            op0=Alu.mult, op1=Alu.add,
        )
        sneg_t = singles.tile([P, B], F32)
        nc.vector.reciprocal(out=sneg_t[:], in_=rn_t[:])

        ga_t = singles.tile([P, B], F32)
        nc.vector.tensor_scalar(
            out=ga_t[:], in0=pms_t[:], scalar1=a_t[:, 0:1], scalar2=None, op0=Alu.mult
        )
        nms_t = singles.tile([P, B], F32)
        nc.vector.tensor_tensor(
            out=nms_t[:], in0=negt_t[:], in1=st_t[:], op=Alu.subtract
        )
        h_t = singles.tile([P, B], F32)
        nc.vector.scalar_tensor_tensor(
            out=h_t[:], in0=nms_t[:], scalar=b_t[:, 0:1], op0=Alu.mult,
            in1=ga_t[:], op1=Alu.subtract,
        )
        g_t = singles.tile([P, B], F32)
        nc.vector.tensor_tensor(
            out=g_t[:], in0=h_t[:], in1=sneg_t[:], op=Alu.mult
        )

        # --- it = inp_trace + inp/10, in place over inp_bc ---
        nc.vector.scalar_tensor_tensor(
            out=inp_bc[:], in0=inp_bc[:], scalar=0.1, op0=Alu.mult,
            in1=tr_bc[:], op1=Alu.add,
        )

        # --- dW[h, :] = g[h] * it  via ScalarE copy with per-partition scale ---
        for b in range(B):
            for j in range(NKCH):
                dw_t = dwpool.tile([P, KCH], F32, tag="dw")
                nc.scalar.activation(
                    out=dw_t[:],
                    in_=inp_bc[:, j * KCH:(j + 1) * KCH],
                    func=Act.Copy,
                    bias=0.0,
                    scale=g_t[:, b:b + 1],
                )
                nc.sync.dma_start(dW_r[:, b, j * KCH:(j + 1) * KCH], dw_t[:])

    nc.compile()
    return nc


def get_nc():
    if "nc" not in _cache:
        _cache["nc"] = _build_nc()
    return _cache["nc"]


def make_in_maps(inputs):
    W = np.asarray(inputs["W"], dtype=np.float32)
    inp = np.ascontiguousarray(np.asarray(inputs["inp"], dtype=np.float32))
    mem = np.asarray(inputs["mem"], dtype=np.float32)
    sptr = np.asarray(inputs["spk_trace"], dtype=np.float32)
    inptr = np.ascontiguousarray(np.asarray(inputs["inp_trace"], dtype=np.float32))
    pred = np.asarray(inputs["prediction"], dtype=np.float32)
    negt = np.asarray(inputs["negative_spk_trace"], dtype=np.float32)
    in_maps = []
    for c in range(NCORES):
        s = slice(c * SH, (c + 1) * SH)
        in_maps.append({
            "W": np.ascontiguousarray(W[s]),
            "inp": inp,
            "mem": np.ascontiguousarray(mem[s]),
            "sptr": np.ascontiguousarray(sptr[s]),
            "inptr": inptr,
            "pred": np.ascontiguousarray(pred[s]),
            "negt": np.ascontiguousarray(negt[s]),
        })
    return in_maps


def assemble(results):
    spk = np.concatenate([np.asarray(results[c]["spk_o"]).reshape(SH) for c in range(NCORES)])
    st = np.concatenate([np.asarray(results[c]["st_o"]).reshape(SH) for c in range(NCORES)])
    dW = np.concatenate(
        [np.asarray(results[c]["dW_o"]).reshape(SH, NIN) for c in range(NCORES)], axis=0
    )
    loss = np.float32(np.asarray(results[0]["loss_o"]).reshape(-1)[0])
    return spk.astype(np.float32), st.astype(np.float32), loss, dW.astype(np.float32)


def run(inputs, trace=False):
    from concourse import bass_utils
    nc = get_nc()
    res = bass_utils.run_bass_kernel_spmd(
        nc, make_in_maps(inputs), list(range(NCORES)), trace=trace
    )
    return res


def kernel(**inputs):
    res = run(inputs, trace=False)
    return assemble(res.results)


# revision 11
# speedup vs baseline: 1.2809x; 1.2809x over previous
"""CLAPP layer step on 8 NeuronCores (Bass/Tile).

Sharding: W / dW / mem / spk_trace / prediction / negative_spk_trace are
row-sharded over the hidden dim (1024 rows per core); inp / inp_trace are
replicated.

Per-core layout: local hidden row h = p*8 + b  (p = SBUF partition 0..127,
b = block 0..7), so [1024] vectors map to [128, 8] tiles with 32-byte
per-partition DMA descriptors and W/dW 128-row blocks are row-strided by 8.

The two data-dependent branch flags (loss > 20, loss_c > -100) are resolved
on the host before dispatch whenever that is provably possible:
  - loss_c = -sum((neg - pred)^2) depends only on inputs, so its flag is
    exact on the host.
  - loss = sum((st - pred)^2) with st = spk_trace + spk/10, spk in {0,1};
    sum(min(...)) / sum(max(...)) over the two spike choices bound it from
    both sides without the matvec.
When both flags are decided, the fast kernel runs: no collective, no
cross-core sync (the scalar losses are returned as per-core partials and
summed on the host), and the dW write stream pipelines directly behind the
W read stream.  If a bound is inconclusive (never for ordinary data), a
fallback kernel computes the flags on-device with an AllReduce.
"""

import numpy as np
from contextlib import ExitStack

NCORES = 8
NH = 8192
NIN = 8192
SH = NH // NCORES          # 1024 rows per core
P = 128
B = SH // P                # 8 blocks
KCH = 4096                 # k-chunk width
NKCH = NIN // KCH          # 2 chunks
BETA = 0.9
THR = 1.0
PI = float(np.pi)

_cache = {}


def _build_nc_fast():
    """Fast path: branch flags are an input, losses leave as partials."""
    from concourse import bacc, tile, mybir

    F32 = mybir.dt.float32
    Alu = mybir.AluOpType
    Act = mybir.ActivationFunctionType

    nc = bacc.Bacc(
        "TRN2",
        target_bir_lowering=False,
        debug=False,
        enable_asserts=False,
        num_devices=NCORES,
    )

    W = nc.dram_tensor("W", [SH, NIN], F32, kind="ExternalInput")
    inp = nc.dram_tensor("inp", [NIN], F32, kind="ExternalInput")
    mem = nc.dram_tensor("mem", [SH], F32, kind="ExternalInput")
    sptr = nc.dram_tensor("sptr", [SH], F32, kind="ExternalInput")
    inptr = nc.dram_tensor("inptr", [NIN], F32, kind="ExternalInput")
    pred = nc.dram_tensor("pred", [SH], F32, kind="ExternalInput")
    negt = nc.dram_tensor("negt", [SH], F32, kind="ExternalInput")
    flags = nc.dram_tensor("flags", [2], F32, kind="ExternalInput")

    spk_o = nc.dram_tensor("spk_o", [SH], F32, kind="ExternalOutput")
    st_o = nc.dram_tensor("st_o", [SH], F32, kind="ExternalOutput")
    lsum_o = nc.dram_tensor("lsum_o", [2], F32, kind="ExternalOutput")
    dW_o = nc.dram_tensor("dW_o", [SH, NIN], F32, kind="ExternalOutput")

    W_r = W.ap().rearrange("(p b) k -> p b k", b=B)
    dW_r = dW_o.ap().rearrange("(p b) k -> p b k", b=B)

    def shard_2d(t):
        return t.ap().rearrange("(p b) -> p b", b=B)

    with tile.TileContext(nc) as tc, ExitStack() as ctx:
        singles = ctx.enter_context(tc.tile_pool(name="singles", bufs=1))
        wpool = ctx.enter_context(tc.tile_pool(name="wpool", bufs=4))
        dwpool = ctx.enter_context(tc.tile_pool(name="dwpool", bufs=3))
        psum = ctx.enter_context(tc.tile_pool(name="psum", bufs=1, space="PSUM"))

        # --- replicated row vectors, broadcast across partitions on GpSimd ---
        inp_bc = singles.tile([P, NIN], F32)
        it_bc = singles.tile([P, NIN], F32)
        nc.sync.dma_start(inp_bc[0:1, :], inp.ap().rearrange("(a k) -> a k", a=1))
        nc.sync.dma_start(it_bc[0:1, :], inptr.ap().rearrange("(a k) -> a k", a=1))
        nc.gpsimd.partition_broadcast(inp_bc[:], inp_bc[0:1, :])
        nc.gpsimd.partition_broadcast(it_bc[:], it_bc[0:1, :])
        # it = inp_trace + inp/10  (in place over the inp_trace broadcast)
        nc.vector.scalar_tensor_tensor(
            out=it_bc[:], in0=inp_bc[:], scalar=0.1, op0=Alu.mult,
            in1=it_bc[:], op1=Alu.add,
        )

        # --- sharded small vectors as [128, 8]; flags broadcast [128, 2] ---
        mem_t = singles.tile([P, B], F32)
        sptr_t = singles.tile([P, B], F32)
        pred_t = singles.tile([P, B], F32)
        negt_t = singles.tile([P, B], F32)
        nc.sync.dma_start(mem_t[:], shard_2d(mem))
        nc.sync.dma_start(sptr_t[:], shard_2d(sptr))
        nc.sync.dma_start(pred_t[:], shard_2d(pred))
        nc.sync.dma_start(negt_t[:], shard_2d(negt))
        fl_bc = singles.tile([P, 2], F32)
        nc.sync.dma_start(
            fl_bc[:], flags.ap().rearrange("(a k) -> a k", a=1).broadcast_to([P, 2])
        )

        ones = singles.tile([P, 1], F32)
        nc.vector.memset(ones[:], 1.0)

        # --- matvec: cur[h] = sum_k W[h, k] * inp[k]; product written in
        # place over the W tile, row sums land in acc ---
        acc = singles.tile([P, B, NKCH], F32)
        for b in range(B):
            for j in range(NKCH):
                w_t = wpool.tile([P, KCH], F32, tag="w")
                nc.sync.dma_start(w_t[:], W_r[:, b, j * KCH:(j + 1) * KCH])
                nc.vector.scalar_tensor_tensor(
                    out=w_t[:],
                    in0=w_t[:],
                    scalar=1.0,
                    op0=Alu.bypass,
                    in1=inp_bc[:, j * KCH:(j + 1) * KCH],
                    op1=Alu.mult,
                    accum_out=acc[:, b, j:j + 1],
                )
        cur_t = singles.tile([P, B], F32)
        nc.vector.tensor_reduce(
            out=cur_t[:], in_=acc[:], axis=mybir.AxisListType.X, op=Alu.add
        )

        # --- LIF neuron step (whole shard, [128, 8]) ---
        rst_t = singles.tile([P, B], F32)
        nc.vector.tensor_scalar(
            out=rst_t[:], in0=mem_t[:], scalar1=THR, scalar2=None, op0=Alu.is_gt
        )
        mn0_t = singles.tile([P, B], F32)
        nc.vector.scalar_tensor_tensor(
            out=mn0_t[:], in0=mem_t[:], scalar=BETA, op0=Alu.mult,
            in1=cur_t[:], op1=Alu.add,
        )
        mnew_t = singles.tile([P, B], F32)
        nc.vector.tensor_tensor(
            out=mnew_t[:], in0=mn0_t[:], in1=rst_t[:], op=Alu.subtract
        )
        spk_t = singles.tile([P, B], F32)
        nc.vector.tensor_scalar(
            out=spk_t[:], in0=mnew_t[:], scalar1=THR, scalar2=None, op0=Alu.is_gt
        )
        st_t = singles.tile([P, B], F32)
        nc.vector.scalar_tensor_tensor(
            out=st_t[:], in0=spk_t[:], scalar=0.1, op0=Alu.mult,
            in1=sptr_t[:], op1=Alu.add,
        )
        nc.gpsimd.dma_start(shard_2d(spk_o), spk_t[:])
        nc.gpsimd.dma_start(shard_2d(st_o), st_t[:])

        # --- loss partials: [sum (pred-st)^2, sum (negt-pred)^2], summed
        # over partitions by a ones-matmul, written out for the host ---
        pms_t = singles.tile([P, B], F32)
        nc.vector.tensor_tensor(
            out=pms_t[:], in0=pred_t[:], in1=st_t[:], op=Alu.subtract
        )
        nmp_t = singles.tile([P, B], F32)
        nc.vector.tensor_tensor(
            out=nmp_t[:], in0=negt_t[:], in1=pred_t[:], op=Alu.subtract
        )
        plos = singles.tile([P, 2], F32)
        sq_scr = singles.tile([P, B], F32)
        nc.vector.scalar_tensor_tensor(
            out=sq_scr[:], in0=pms_t[:], scalar=1.0, op0=Alu.bypass,
            in1=pms_t[:], op1=Alu.mult, accum_out=plos[:, 0:1],
        )
        sq_scr2 = singles.tile([P, B], F32)
        nc.vector.scalar_tensor_tensor(
            out=sq_scr2[:], in0=nmp_t[:], scalar=1.0, op0=Alu.bypass,
            in1=nmp_t[:], op1=Alu.mult, accum_out=plos[:, 1:2],
        )
        lsum_ps = psum.tile([1, 2], F32)
        nc.tensor.matmul(lsum_ps[:], ones[:], plos[:])
        lsum_sb = singles.tile([1, 2], F32)
        nc.vector.tensor_copy(lsum_sb[:], lsum_ps[:])
        nc.gpsimd.dma_start(lsum_o.ap().rearrange("(a k) -> a k", a=1), lsum_sb[:])

        # --- g = (a*(pred-st) - b*(negt-st)) * surr with host-decided a, b ---
        negpi_t = singles.tile([P, 1], F32)
        nc.vector.memset(negpi_t[:], -PI)
        ssq_t = singles.tile([P, B], F32)
        nc.scalar.activation(
            out=ssq_t[:], in_=mnew_t[:], func=Act.Square, bias=negpi_t[:, 0:1],
            scale=PI,
        )
        rn_t = singles.tile([P, B], F32)
        nc.vector.tensor_scalar(
            out=rn_t[:], in0=ssq_t[:], scalar1=PI, scalar2=PI,
            op0=Alu.mult, op1=Alu.add,
        )
        surr_t = singles.tile([P, B], F32)
        nc.vector.reciprocal(out=surr_t[:], in_=rn_t[:])

        ga_t = singles.tile([P, B], F32)
        nc.vector.tensor_scalar(
            out=ga_t[:], in0=pms_t[:], scalar1=fl_bc[:, 0:1], scalar2=None,
            op0=Alu.mult,
        )
        nms_t = singles.tile([P, B], F32)
        nc.vector.tensor_tensor(
            out=nms_t[:], in0=negt_t[:], in1=st_t[:], op=Alu.subtract
        )
        hb_t = singles.tile([P, B], F32)
        nc.vector.scalar_tensor_tensor(
            out=hb_t[:], in0=nms_t[:], scalar=fl_bc[:, 1:2], op0=Alu.mult,
            in1=ga_t[:], op1=Alu.subtract,
        )
        g_t = singles.tile([P, B], F32)
        nc.vector.tensor_tensor(
            out=g_t[:], in0=hb_t[:], in1=surr_t[:], op=Alu.mult
        )
        nc.vector.tensor_scalar(
            out=g_t[:], in0=g_t[:], scalar1=-1.0, scalar2=None, op0=Alu.mult
        )

        # --- dW[h, :] = g[h] * it, streamed out right behind the reads ---
        for b in range(B):
            for j in range(NKCH):
                dw_t = dwpool.tile([P, KCH], F32, tag="dw")
                nc.scalar.activation(
                    out=dw_t[:],
                    in_=it_bc[:, j * KCH:(j + 1) * KCH],
                    func=Act.Copy,
                    bias=0.0,
                    scale=g_t[:, b:b + 1],
                )
                nc.scalar.dma_start(dW_r[:, b, j * KCH:(j + 1) * KCH], dw_t[:])

    nc.compile()
    return nc


def _build_nc_fallback():
    """General path: flags computed on-device via an AllReduce of the
    losses.  Used only when the host-side bounds cannot decide a flag."""
    from concourse import bacc, tile, mybir

    F32 = mybir.dt.float32
    Alu = mybir.AluOpType
    Act = mybir.ActivationFunctionType

    nc = bacc.Bacc(
        "TRN2",
        target_bir_lowering=False,
        debug=False,
        enable_asserts=False,
        num_devices=NCORES,
    )

    W = nc.dram_tensor("W", [SH, NIN], F32, kind="ExternalInput")
    inp = nc.dram_tensor("inp", [NIN], F32, kind="ExternalInput")
    mem = nc.dram_tensor("mem", [SH], F32, kind="ExternalInput")
    sptr = nc.dram_tensor("sptr", [SH], F32, kind="ExternalInput")
    inptr = nc.dram_tensor("inptr", [NIN], F32, kind="ExternalInput")
    pred = nc.dram_tensor("pred", [SH], F32, kind="ExternalInput")
    negt = nc.dram_tensor("negt", [SH], F32, kind="ExternalInput")

    spk_o = nc.dram_tensor("spk_o", [SH], F32, kind="ExternalOutput")
    st_o = nc.dram_tensor("st_o", [SH], F32, kind="ExternalOutput")
    loss_o = nc.dram_tensor("loss_o", [1], F32, kind="ExternalOutput")
    dW_o = nc.dram_tensor("dW_o", [SH, NIN], F32, kind="ExternalOutput")

    W_r = W.ap().rearrange("(p b) k -> p b k", b=B)
    dW_r = dW_o.ap().rearrange("(p b) k -> p b k", b=B)

    def shard_2d(t):
        return t.ap().rearrange("(p b) -> p b", b=B)

    with tile.TileContext(nc) as tc, ExitStack() as ctx:
        singles = ctx.enter_context(tc.tile_pool(name="singles", bufs=1))
        wpool = ctx.enter_context(tc.tile_pool(name="wpool", bufs=3))
        dwpool = ctx.enter_context(tc.tile_pool(name="dwpool", bufs=3))
        psum = ctx.enter_context(tc.tile_pool(name="psum", bufs=1, space="PSUM"))
        dram = ctx.enter_context(tc.tile_pool(name="dram", bufs=1, space="DRAM"))

        inp_bc = singles.tile([P, NIN], F32)
        tr_bc = singles.tile([P, NIN], F32)
        nc.sync.dma_start(
            inp_bc[:], inp.ap().rearrange("(a k) -> a k", a=1).broadcast_to([P, NIN])
        )
        nc.sync.dma_start(
            tr_bc[:], inptr.ap().rearrange("(a k) -> a k", a=1).broadcast_to([P, NIN])
        )

        mem_t = singles.tile([P, B], F32)
        sptr_t = singles.tile([P, B], F32)
        pred_t = singles.tile([P, B], F32)
        negt_t = singles.tile([P, B], F32)
        nc.sync.dma_start(mem_t[:], shard_2d(mem))
        nc.sync.dma_start(sptr_t[:], shard_2d(sptr))
        nc.sync.dma_start(pred_t[:], shard_2d(pred))
        nc.sync.dma_start(negt_t[:], shard_2d(negt))

        ones = singles.tile([P, 1], F32)
        nc.vector.memset(ones[:], 1.0)

        acc = singles.tile([P, B, NKCH], F32)
        scr = singles.tile([P, KCH], F32)
        for b in range(B):
            for j in range(NKCH):
                w_t = wpool.tile([P, KCH], F32, tag="w")
                nc.sync.dma_start(w_t[:], W_r[:, b, j * KCH:(j + 1) * KCH])
                nc.vector.scalar_tensor_tensor(
                    out=scr[:],
                    in0=w_t[:],
                    scalar=1.0,
                    op0=Alu.bypass,
                    in1=inp_bc[:, j * KCH:(j + 1) * KCH],
                    op1=Alu.mult,
                    accum_out=acc[:, b, j:j + 1],
                )
        cur_t = singles.tile([P, B], F32)
        nc.vector.tensor_reduce(
            out=cur_t[:], in_=acc[:], axis=mybir.AxisListType.X, op=Alu.add
        )

        rst_t = singles.tile([P, B], F32)
        nc.vector.tensor_scalar(
            out=rst_t[:], in0=mem_t[:], scalar1=THR, scalar2=None, op0=Alu.is_gt
        )
        mn0_t = singles.tile([P, B], F32)
        nc.vector.scalar_tensor_tensor(
            out=mn0_t[:], in0=mem_t[:], scalar=BETA, op0=Alu.mult,
            in1=cur_t[:], op1=Alu.add,
        )
        mnew_t = singles.tile([P, B], F32)
        nc.vector.tensor_tensor(
            out=mnew_t[:], in0=mn0_t[:], in1=rst_t[:], op=Alu.subtract
        )
        spk_t = singles.tile([P, B], F32)
        nc.vector.tensor_scalar(
            out=spk_t[:], in0=mnew_t[:], scalar1=THR, scalar2=None, op0=Alu.is_gt
        )
        st_t = singles.tile([P, B], F32)
        nc.vector.scalar_tensor_tensor(
            out=st_t[:], in0=spk_t[:], scalar=0.1, op0=Alu.mult,
            in1=sptr_t[:], op1=Alu.add,
        )
        nc.sync.dma_start(shard_2d(spk_o), spk_t[:])
        nc.sync.dma_start(shard_2d(st_o), st_t[:])

        pms_t = singles.tile([P, B], F32)
        nc.vector.tensor_tensor(
            out=pms_t[:], in0=pred_t[:], in1=st_t[:], op=Alu.subtract
        )
        nmp_t = singles.tile([P, B], F32)
        nc.vector.tensor_tensor(
            out=nmp_t[:], in0=negt_t[:], in1=pred_t[:], op=Alu.subtract
        )
        plos = singles.tile([P, 2], F32)
        sq_scr = singles.tile([P, B], F32)
        nc.vector.scalar_tensor_tensor(
            out=sq_scr[:], in0=pms_t[:], scalar=1.0, op0=Alu.bypass,
            in1=pms_t[:], op1=Alu.mult, accum_out=plos[:, 0:1],
        )
        sq_scr2 = singles.tile([P, B], F32)
        nc.vector.scalar_tensor_tensor(
            out=sq_scr2[:], in0=nmp_t[:], scalar=1.0, op0=Alu.bypass,
            in1=nmp_t[:], op1=Alu.mult, accum_out=plos[:, 1:2],
        )
        lsum_ps = psum.tile([1, 2], F32)
        nc.tensor.matmul(lsum_ps[:], ones[:], plos[:])
        lsum_sb = singles.tile([1, 2], F32)
        nc.vector.tensor_copy(lsum_sb[:], lsum_ps[:])

        cc_in = dram.tile([1, 2], F32)
        cc_out = dram.tile([1, 2], F32)
        nc.gpsimd.dma_start(cc_in[:], lsum_sb[:])
        nc.gpsimd.collective_compute(
            "AllReduce",
            Alu.add,
            replica_groups=[list(range(NCORES))],
            ins=[cc_in.opt()],
            outs=[cc_out.opt()],
        )
        ar_bc = singles.tile([P, 2], F32)
        nc.sync.dma_start(ar_bc[:], cc_out[:].broadcast_to([P, 2]))

        ld = singles.tile([1, 1], F32)
        nc.vector.tensor_tensor(
            out=ld[:], in0=ar_bc[0:1, 0:1], in1=ar_bc[0:1, 1:2], op=Alu.subtract
        )
        nc.vector.tensor_scalar(
            out=ld[:], in0=ld[:], scalar1=0.5, scalar2=None, op0=Alu.mult
        )
        nc.sync.dma_start(loss_o.ap().rearrange("(a k) -> a k", a=1), ld[:])

        a_t = singles.tile([P, 1], F32)
        nc.vector.tensor_scalar(
            out=a_t[:], in0=ar_bc[:, 0:1], scalar1=20.0, scalar2=None, op0=Alu.is_gt
        )
        b_t = singles.tile([P, 1], F32)
        nc.vector.tensor_scalar(
            out=b_t[:], in0=ar_bc[:, 1:2], scalar1=100.0, scalar2=None, op0=Alu.is_lt
        )
        negpi_t = singles.tile([P, 1], F32)
        nc.vector.memset(negpi_t[:], -PI)
        ssq_t = singles.tile([P, B], F32)
        nc.scalar.activation(
            out=ssq_t[:], in_=mnew_t[:], func=Act.Square, bias=negpi_t[:, 0:1],
            scale=PI,
        )
        rn_t = singles.tile([P, B], F32)
        nc.vector.tensor_scalar(
            out=rn_t[:], in0=ssq_t[:], scalar1=-PI, scalar2=-PI,
            op0=Alu.mult, op1=Alu.add,
        )
        sneg_t = singles.tile([P, B], F32)
        nc.vector.reciprocal(out=sneg_t[:], in_=rn_t[:])

        ga_t = singles.tile([P, B], F32)
        nc.vector.tensor_scalar(
            out=ga_t[:], in0=pms_t[:], scalar1=a_t[:, 0:1], scalar2=None, op0=Alu.mult
        )
        nms_t = singles.tile([P, B], F32)
        nc.vector.tensor_tensor(
            out=nms_t[:], in0=negt_t[:], in1=st_t[:], op=Alu.subtract
        )
        h_t = singles.tile([P, B], F32)
        nc.vector.scalar_tensor_tensor(
            out=h_t[:], in0=nms_t[:], scalar=b_t[:, 0:1], op0=Alu.mult,
            in1=ga_t[:], op1=Alu.subtract,
        )
        g_t = singles.tile([P, B], F32)
        nc.vector.tensor_tensor(
            out=g_t[:], in0=h_t[:], in1=sneg_t[:], op=Alu.mult
        )

        nc.vector.scalar_tensor_tensor(
            out=inp_bc[:], in0=inp_bc[:], scalar=0.1, op0=Alu.mult,
            in1=tr_bc[:], op1=Alu.add,
        )

        for b in range(B):
            for j in range(NKCH):
                dw_t = dwpool.tile([P, KCH], F32, tag="dw")
                nc.scalar.activation(
                    out=dw_t[:],
                    in_=inp_bc[:, j * KCH:(j + 1) * KCH],
                    func=Act.Copy,
                    bias=0.0,
                    scale=g_t[:, b:b + 1],
                )
                nc.sync.dma_start(dW_r[:, b, j * KCH:(j + 1) * KCH], dw_t[:])

    nc.compile()
    return nc


def get_nc(kind="fast"):
    key = "nc_" + kind
    if key not in _cache:
        _cache[key] = _build_nc_fast() if kind == "fast" else _build_nc_fallback()
    return _cache[key]


def decide_flags(inputs):
    """Resolve the two branch flags on the host when provably possible.
    Returns (a, b) floats or None if a bound is inconclusive."""
    pred = np.asarray(inputs["prediction"], np.float64)
    negt = np.asarray(inputs["negative_spk_trace"], np.float64)
    sptr = np.asarray(inputs["spk_trace"], np.float64)

    S = float(np.sum((negt - pred) ** 2))
    # guard against fp32-accumulation flips near the threshold
    eps_s = 1e-3 * S + 1e-2
    if S > 100.0 + eps_s:
        b = 0.0
    elif S < 100.0 - eps_s:
        b = 1.0
    else:
        return None

    d0 = (sptr - pred) ** 2
    d1 = (sptr + 0.1 - pred) ** 2
    lmin = float(np.sum(np.minimum(d0, d1)))
    lmax = float(np.sum(np.maximum(d0, d1)))
    eps_l = 1e-3 * max(lmin, lmax) + 1e-2
    if lmin > 20.0 + eps_l:
        a = 1.0
    elif lmax < 20.0 - eps_l:
        a = 0.0
    else:
        return None
    return a, b


def make_in_maps(inputs, flags=None):
    W = np.asarray(inputs["W"], dtype=np.float32)
    inp = np.ascontiguousarray(np.asarray(inputs["inp"], dtype=np.float32))
    mem = np.asarray(inputs["mem"], dtype=np.float32)
    sptr = np.asarray(inputs["spk_trace"], dtype=np.float32)
    inptr = np.ascontiguousarray(np.asarray(inputs["inp_trace"], dtype=np.float32))
    pred = np.asarray(inputs["prediction"], dtype=np.float32)
    negt = np.asarray(inputs["negative_spk_trace"], dtype=np.float32)
    in_maps = []
    for c in range(NCORES):
        s = slice(c * SH, (c + 1) * SH)
        m = {
            "W": np.ascontiguousarray(W[s]),
            "inp": inp,
            "mem": np.ascontiguousarray(mem[s]),
            "sptr": np.ascontiguousarray(sptr[s]),
            "inptr": inptr,
            "pred": np.ascontiguousarray(pred[s]),
            "negt": np.ascontiguousarray(negt[s]),
        }
        if flags is not None:
            m["flags"] = np.asarray(flags, dtype=np.float32)
        in_maps.append(m)
    return in_maps


def assemble_fast(results):
    spk = np.concatenate([np.asarray(results[c]["spk_o"]).reshape(SH) for c in range(NCORES)])
    st = np.concatenate([np.asarray(results[c]["st_o"]).reshape(SH) for c in range(NCORES)])
    dW = np.concatenate(
        [np.asarray(results[c]["dW_o"]).reshape(SH, NIN) for c in range(NCORES)], axis=0
    )
    L = np.float32(0.0)
    S = np.float32(0.0)
    for c in range(NCORES):
        ls = np.asarray(results[c]["lsum_o"]).reshape(2).astype(np.float32)
        L = np.float32(L + ls[0])
        S = np.float32(S + ls[1])
    loss = np.float32((L - S) / np.float32(2.0))
    return spk.astype(np.float32), st.astype(np.float32), loss, dW.astype(np.float32)


def assemble_fallback(results):
    spk = np.concatenate([np.asarray(results[c]["spk_o"]).reshape(SH) for c in range(NCORES)])
    st = np.concatenate([np.asarray(results[c]["st_o"]).reshape(SH) for c in range(NCORES)])
    dW = np.concatenate(
        [np.asarray(results[c]["dW_o"]).reshape(SH, NIN) for c in range(NCORES)], axis=0
    )
    loss = np.float32(np.asarray(results[0]["loss_o"]).reshape(-1)[0])
    return spk.astype(np.float32), st.astype(np.float32), loss, dW.astype(np.float32)


def run(inputs, trace=False, trace_cores=None):
    from concourse import bass_utils
    flags = decide_flags(inputs)
    if flags is not None:
        nc = get_nc("fast")
        res = bass_utils.run_bass_kernel_spmd(
            nc, make_in_maps(inputs, flags=flags), list(range(NCORES)),
            trace=trace, trace_cores=trace_cores,
        )
        return res, "fast"
    nc = get_nc("fallback")
    res = bass_utils.run_bass_kernel_spmd(
        nc, make_in_maps(inputs), list(range(NCORES)),
        trace=trace, trace_cores=trace_cores,
    )
    return res, "fallback"


def kernel(**inputs):
    res, kind = run(inputs, trace=False)
    if kind == "fast":
        return assemble_fast(res.results)
    return assemble_fallback(res.results)


# revision 12
# speedup vs baseline: 1.3040x; 1.0180x over previous
"""CLAPP layer step on 8 NeuronCores (Bass/Tile).

Sharding: W / dW / mem / spk_trace / prediction / negative_spk_trace are
row-sharded over the hidden dim (1024 rows per core); inp / inp_trace are
replicated.

Per-core layout: local hidden row h = p*8 + b  (p = SBUF partition 0..127,
b = block 0..7), so [1024] vectors map to [128, 8] tiles with 32-byte
per-partition DMA descriptors and W/dW 128-row blocks are row-strided by 8.

The two data-dependent branch flags (loss > 20, loss_c > -100) are resolved
on the host before dispatch whenever that is provably possible:
  - loss_c = -sum((neg - pred)^2) depends only on inputs, so its flag is
    exact on the host.
  - loss = sum((st - pred)^2) with st = spk_trace + spk/10, spk in {0,1};
    sum(min(...)) / sum(max(...)) over the two spike choices bound it from
    both sides without the matvec.
When both flags are decided, the fast kernel runs: no collective, no
cross-core sync (the scalar losses are returned as per-core partials and
summed on the host), and the dW write stream pipelines directly behind the
W read stream.  If a bound is inconclusive (never for ordinary data), a
fallback kernel computes the flags on-device with an AllReduce.
"""

import numpy as np
from contextlib import ExitStack

NCORES = 8
NH = 8192
NIN = 8192
SH = NH // NCORES          # 1024 rows per core
P = 128
B = SH // P                # 8 blocks
KCH = 4096                 # k-chunk width
NKCH = NIN // KCH          # 2 chunks
BETA = 0.9
THR = 1.0
PI = float(np.pi)

_cache = {}


def _build_nc_fast():
    """Fast path: branch flags are an input, losses leave as partials."""
    from concourse import bacc, tile, mybir

    F32 = mybir.dt.float32
    Alu = mybir.AluOpType
    Act = mybir.ActivationFunctionType

    nc = bacc.Bacc(
        "TRN2",
        target_bir_lowering=False,
        debug=False,
        enable_asserts=False,
        num_devices=NCORES,
    )

    W = nc.dram_tensor("W", [SH, NIN], F32, kind="ExternalInput")
    inp = nc.dram_tensor("inp", [NIN], F32, kind="ExternalInput")
    mem = nc.dram_tensor("mem", [SH], F32, kind="ExternalInput")
    sptr = nc.dram_tensor("sptr", [SH], F32, kind="ExternalInput")
    inptr = nc.dram_tensor("inptr", [NIN], F32, kind="ExternalInput")
    pred = nc.dram_tensor("pred", [SH], F32, kind="ExternalInput")
    negt = nc.dram_tensor("negt", [SH], F32, kind="ExternalInput")
    flags = nc.dram_tensor("flags", [2], F32, kind="ExternalInput")

    spk_o = nc.dram_tensor("spk_o", [SH], F32, kind="ExternalOutput")
    st_o = nc.dram_tensor("st_o", [SH], F32, kind="ExternalOutput")
    lsum_o = nc.dram_tensor("lsum_o", [2], F32, kind="ExternalOutput")
    dW_o = nc.dram_tensor("dW_o", [SH, NIN], F32, kind="ExternalOutput")

    W_r = W.ap().rearrange("(p b) k -> p b k", b=B)
    dW_r = dW_o.ap().rearrange("(p b) k -> p b k", b=B)

    def shard_2d(t):
        return t.ap().rearrange("(p b) -> p b", b=B)

    with tile.TileContext(nc) as tc, ExitStack() as ctx:
        singles = ctx.enter_context(tc.tile_pool(name="singles", bufs=1))
        wpool = ctx.enter_context(tc.tile_pool(name="wpool", bufs=4))
        dwpool = ctx.enter_context(tc.tile_pool(name="dwpool", bufs=3))
        psum = ctx.enter_context(tc.tile_pool(name="psum", bufs=1, space="PSUM"))

        # --- inp broadcast across partitions on GpSimd (matvec needs it) ---
        inp_bc = singles.tile([P, NIN], F32)
        it_bc = singles.tile([P, NIN], F32)
        nc.sync.dma_start(inp_bc[0:1, :], inp.ap().rearrange("(a k) -> a k", a=1))
        nc.gpsimd.partition_broadcast(inp_bc[:], inp_bc[0:1, :])

        # --- matvec: cur[h] = sum_k W[h, k] * inp[k]; product written in
        # place over the W tile, row sums land in acc ---
        acc = singles.tile([P, B, NKCH], F32)
        for b in range(B):
            for j in range(NKCH):
                w_t = wpool.tile([P, KCH], F32, tag="w")
                nc.sync.dma_start(w_t[:], W_r[:, b, j * KCH:(j + 1) * KCH])
                nc.vector.scalar_tensor_tensor(
                    out=w_t[:],
                    in0=w_t[:],
                    scalar=1.0,
                    op0=Alu.bypass,
                    in1=inp_bc[:, j * KCH:(j + 1) * KCH],
                    op1=Alu.mult,
                    accum_out=acc[:, b, j:j + 1],
                )

        # --- it = inp_trace + inp/10, broadcast (needed only by the dW
        # stream; emitted after the matvec so the scheduler keeps the DVE
        # free for it) ---
        nc.sync.dma_start(it_bc[0:1, :], inptr.ap().rearrange("(a k) -> a k", a=1))
        nc.gpsimd.partition_broadcast(it_bc[:], it_bc[0:1, :])
        nc.vector.scalar_tensor_tensor(
            out=it_bc[:], in0=inp_bc[:], scalar=0.1, op0=Alu.mult,
            in1=it_bc[:], op1=Alu.add,
        )

        # --- sharded small vectors as [128, 8]; flags broadcast [128, 2] ---
        mem_t = singles.tile([P, B], F32)
        sptr_t = singles.tile([P, B], F32)
        pred_t = singles.tile([P, B], F32)
        negt_t = singles.tile([P, B], F32)
        nc.sync.dma_start(mem_t[:], shard_2d(mem))
        nc.sync.dma_start(sptr_t[:], shard_2d(sptr))
        nc.sync.dma_start(pred_t[:], shard_2d(pred))
        nc.sync.dma_start(negt_t[:], shard_2d(negt))
        fl_bc = singles.tile([P, 2], F32)
        nc.sync.dma_start(
            fl_bc[:], flags.ap().rearrange("(a k) -> a k", a=1).broadcast_to([P, 2])
        )

        ones = singles.tile([P, 1], F32)
        nc.vector.memset(ones[:], 1.0)

        cur_t = singles.tile([P, B], F32)
        nc.vector.tensor_reduce(
            out=cur_t[:], in_=acc[:], axis=mybir.AxisListType.X, op=Alu.add
        )

        # --- LIF neuron step (whole shard, [128, 8]) ---
        rst_t = singles.tile([P, B], F32)
        nc.vector.tensor_scalar(
            out=rst_t[:], in0=mem_t[:], scalar1=THR, scalar2=None, op0=Alu.is_gt
        )
        mn0_t = singles.tile([P, B], F32)
        nc.vector.scalar_tensor_tensor(
            out=mn0_t[:], in0=mem_t[:], scalar=BETA, op0=Alu.mult,
            in1=cur_t[:], op1=Alu.add,
        )
        mnew_t = singles.tile([P, B], F32)
        nc.vector.tensor_tensor(
            out=mnew_t[:], in0=mn0_t[:], in1=rst_t[:], op=Alu.subtract
        )
        spk_t = singles.tile([P, B], F32)
        nc.vector.tensor_scalar(
            out=spk_t[:], in0=mnew_t[:], scalar1=THR, scalar2=None, op0=Alu.is_gt
        )
        st_t = singles.tile([P, B], F32)
        nc.vector.scalar_tensor_tensor(
            out=st_t[:], in0=spk_t[:], scalar=0.1, op0=Alu.mult,
            in1=sptr_t[:], op1=Alu.add,
        )
        nc.gpsimd.dma_start(shard_2d(spk_o), spk_t[:])
        nc.gpsimd.dma_start(shard_2d(st_o), st_t[:])

        # --- loss partials: [sum (pred-st)^2, sum (negt-pred)^2], summed
        # over partitions by a ones-matmul, written out for the host ---
        pms_t = singles.tile([P, B], F32)
        nc.vector.tensor_tensor(
            out=pms_t[:], in0=pred_t[:], in1=st_t[:], op=Alu.subtract
        )
        nmp_t = singles.tile([P, B], F32)
        nc.vector.tensor_tensor(
            out=nmp_t[:], in0=negt_t[:], in1=pred_t[:], op=Alu.subtract
        )
        plos = singles.tile([P, 2], F32)
        sq_scr = singles.tile([P, B], F32)
        nc.vector.scalar_tensor_tensor(
            out=sq_scr[:], in0=pms_t[:], scalar=1.0, op0=Alu.bypass,
            in1=pms_t[:], op1=Alu.mult, accum_out=plos[:, 0:1],
        )
        sq_scr2 = singles.tile([P, B], F32)
        nc.vector.scalar_tensor_tensor(
            out=sq_scr2[:], in0=nmp_t[:], scalar=1.0, op0=Alu.bypass,
            in1=nmp_t[:], op1=Alu.mult, accum_out=plos[:, 1:2],
        )
        lsum_ps = psum.tile([1, 2], F32)
        nc.tensor.matmul(lsum_ps[:], ones[:], plos[:])
        lsum_sb = singles.tile([1, 2], F32)
        nc.vector.tensor_copy(lsum_sb[:], lsum_ps[:])
        nc.gpsimd.dma_start(lsum_o.ap().rearrange("(a k) -> a k", a=1), lsum_sb[:])

        # --- g = (a*(pred-st) - b*(negt-st)) * surr with host-decided a, b ---
        negpi_t = singles.tile([P, 1], F32)
        nc.vector.memset(negpi_t[:], -PI)
        ssq_t = singles.tile([P, B], F32)
        nc.scalar.activation(
            out=ssq_t[:], in_=mnew_t[:], func=Act.Square, bias=negpi_t[:, 0:1],
            scale=PI,
        )
        rn_t = singles.tile([P, B], F32)
        nc.vector.tensor_scalar(
            out=rn_t[:], in0=ssq_t[:], scalar1=PI, scalar2=PI,
            op0=Alu.mult, op1=Alu.add,
        )
        surr_t = singles.tile([P, B], F32)
        nc.vector.reciprocal(out=surr_t[:], in_=rn_t[:])

        ga_t = singles.tile([P, B], F32)
        nc.vector.tensor_scalar(
            out=ga_t[:], in0=pms_t[:], scalar1=fl_bc[:, 0:1], scalar2=None,
            op0=Alu.mult,
        )
        nms_t = singles.tile([P, B], F32)
        nc.vector.tensor_tensor(
            out=nms_t[:], in0=negt_t[:], in1=st_t[:], op=Alu.subtract
        )
        hb_t = singles.tile([P, B], F32)
        nc.vector.scalar_tensor_tensor(
            out=hb_t[:], in0=nms_t[:], scalar=fl_bc[:, 1:2], op0=Alu.mult,
            in1=ga_t[:], op1=Alu.subtract,
        )
        g_t = singles.tile([P, B], F32)
        nc.vector.tensor_tensor(
            out=g_t[:], in0=hb_t[:], in1=surr_t[:], op=Alu.mult
        )
        nc.vector.tensor_scalar(
            out=g_t[:], in0=g_t[:], scalar1=-1.0, scalar2=None, op0=Alu.mult
        )

        # --- dW[h, :] = g[h] * it, streamed out right behind the reads ---
        for b in range(B):
            for j in range(NKCH):
                dw_t = dwpool.tile([P, KCH], F32, tag="dw")
                nc.scalar.activation(
                    out=dw_t[:],
                    in_=it_bc[:, j * KCH:(j + 1) * KCH],
                    func=Act.Copy,
                    bias=0.0,
                    scale=g_t[:, b:b + 1],
                )
                nc.scalar.dma_start(dW_r[:, b, j * KCH:(j + 1) * KCH], dw_t[:])

    nc.compile()
    return nc


def _build_nc_fallback():
    """General path: flags computed on-device via an AllReduce of the
    losses.  Used only when the host-side bounds cannot decide a flag."""
    from concourse import bacc, tile, mybir

    F32 = mybir.dt.float32
    Alu = mybir.AluOpType
    Act = mybir.ActivationFunctionType

    nc = bacc.Bacc(
        "TRN2",
        target_bir_lowering=False,
        debug=False,
        enable_asserts=False,
        num_devices=NCORES,
    )

    W = nc.dram_tensor("W", [SH, NIN], F32, kind="ExternalInput")
    inp = nc.dram_tensor("inp", [NIN], F32, kind="ExternalInput")
    mem = nc.dram_tensor("mem", [SH], F32, kind="ExternalInput")
    sptr = nc.dram_tensor("sptr", [SH], F32, kind="ExternalInput")
    inptr = nc.dram_tensor("inptr", [NIN], F32, kind="ExternalInput")
    pred = nc.dram_tensor("pred", [SH], F32, kind="ExternalInput")
    negt = nc.dram_tensor("negt", [SH], F32, kind="ExternalInput")

    spk_o = nc.dram_tensor("spk_o", [SH], F32, kind="ExternalOutput")
    st_o = nc.dram_tensor("st_o", [SH], F32, kind="ExternalOutput")
    loss_o = nc.dram_tensor("loss_o", [1], F32, kind="ExternalOutput")
    dW_o = nc.dram_tensor("dW_o", [SH, NIN], F32, kind="ExternalOutput")

    W_r = W.ap().rearrange("(p b) k -> p b k", b=B)
    dW_r = dW_o.ap().rearrange("(p b) k -> p b k", b=B)

    def shard_2d(t):
        return t.ap().rearrange("(p b) -> p b", b=B)

    with tile.TileContext(nc) as tc, ExitStack() as ctx:
        singles = ctx.enter_context(tc.tile_pool(name="singles", bufs=1))
        wpool = ctx.enter_context(tc.tile_pool(name="wpool", bufs=3))
        dwpool = ctx.enter_context(tc.tile_pool(name="dwpool", bufs=3))
        psum = ctx.enter_context(tc.tile_pool(name="psum", bufs=1, space="PSUM"))
        dram = ctx.enter_context(tc.tile_pool(name="dram", bufs=1, space="DRAM"))

        inp_bc = singles.tile([P, NIN], F32)
        tr_bc = singles.tile([P, NIN], F32)
        nc.sync.dma_start(
            inp_bc[:], inp.ap().rearrange("(a k) -> a k", a=1).broadcast_to([P, NIN])
        )
        nc.sync.dma_start(
            tr_bc[:], inptr.ap().rearrange("(a k) -> a k", a=1).broadcast_to([P, NIN])
        )

        mem_t = singles.tile([P, B], F32)
        sptr_t = singles.tile([P, B], F32)
        pred_t = singles.tile([P, B], F32)
        negt_t = singles.tile([P, B], F32)
        nc.sync.dma_start(mem_t[:], shard_2d(mem))
        nc.sync.dma_start(sptr_t[:], shard_2d(sptr))
        nc.sync.dma_start(pred_t[:], shard_2d(pred))
        nc.sync.dma_start(negt_t[:], shard_2d(negt))

        ones = singles.tile([P, 1], F32)
        nc.vector.memset(ones[:], 1.0)

        acc = singles.tile([P, B, NKCH], F32)
        scr = singles.tile([P, KCH], F32)
        for b in range(B):
            for j in range(NKCH):
                w_t = wpool.tile([P, KCH], F32, tag="w")
                nc.sync.dma_start(w_t[:], W_r[:, b, j * KCH:(j + 1) * KCH])
                nc.vector.scalar_tensor_tensor(
                    out=scr[:],
                    in0=w_t[:],
                    scalar=1.0,
                    op0=Alu.bypass,
                    in1=inp_bc[:, j * KCH:(j + 1) * KCH],
                    op1=Alu.mult,
                    accum_out=acc[:, b, j:j + 1],
                )
        cur_t = singles.tile([P, B], F32)
        nc.vector.tensor_reduce(
            out=cur_t[:], in_=acc[:], axis=mybir.AxisListType.X, op=Alu.add
        )

        rst_t = singles.tile([P, B], F32)
        nc.vector.tensor_scalar(
            out=rst_t[:], in0=mem_t[:], scalar1=THR, scalar2=None, op0=Alu.is_gt
        )
        mn0_t = singles.tile([P, B], F32)
        nc.vector.scalar_tensor_tensor(
            out=mn0_t[:], in0=mem_t[:], scalar=BETA, op0=Alu.mult,
            in1=cur_t[:], op1=Alu.add,
        )
        mnew_t = singles.tile([P, B], F32)
        nc.vector.tensor_tensor(
            out=mnew_t[:], in0=mn0_t[:], in1=rst_t[:], op=Alu.subtract
        )
        spk_t = singles.tile([P, B], F32)
        nc.vector.tensor_scalar(
            out=spk_t[:], in0=mnew_t[:], scalar1=THR, scalar2=None, op0=Alu.is_gt
        )
        st_t = singles.tile([P, B], F32)
        nc.vector.scalar_tensor_tensor(
            out=st_t[:], in0=spk_t[:], scalar=0.1, op0=Alu.mult,
            in1=sptr_t[:], op1=Alu.add,
        )
        nc.sync.dma_start(shard_2d(spk_o), spk_t[:])
        nc.sync.dma_start(shard_2d(st_o), st_t[:])

        pms_t = singles.tile([P, B], F32)
        nc.vector.tensor_tensor(
            out=pms_t[:], in0=pred_t[:], in1=st_t[:], op=Alu.subtract
        )
        nmp_t = singles.tile([P, B], F32)
        nc.vector.tensor_tensor(
            out=nmp_t[:], in0=negt_t[:], in1=pred_t[:], op=Alu.subtract
        )
        plos = singles.tile([P, 2], F32)
        sq_scr = singles.tile([P, B], F32)
        nc.vector.scalar_tensor_tensor(
            out=sq_scr[:], in0=pms_t[:], scalar=1.0, op0=Alu.bypass,
            in1=pms_t[:], op1=Alu.mult, accum_out=plos[:, 0:1],
        )
        sq_scr2 = singles.tile([P, B], F32)
        nc.vector.scalar_tensor_tensor(
            out=sq_scr2[:], in0=nmp_t[:], scalar=1.0, op0=Alu.bypass,
            in1=nmp_t[:], op1=Alu.mult, accum_out=plos[:, 1:2],
        )
        lsum_ps = psum.tile([1, 2], F32)
        nc.tensor.matmul(lsum_ps[:], ones[:], plos[:])
        lsum_sb = singles.tile([1, 2], F32)
        nc.vector.tensor_copy(lsum_sb[:], lsum_ps[:])

        cc_in = dram.tile([1, 2], F32)
        cc_out = dram.tile([1, 2], F32)
        nc.gpsimd.dma_start(cc_in[:], lsum_sb[:])
        nc.gpsimd.collective_compute(
            "AllReduce",
            Alu.add,
            replica_groups=[list(range(NCORES))],
            ins=[cc_in.opt()],
            outs=[cc_out.opt()],
        )
        ar_bc = singles.tile([P, 2], F32)
        nc.sync.dma_start(ar_bc[:], cc_out[:].broadcast_to([P, 2]))

        ld = singles.tile([1, 1], F32)
        nc.vector.tensor_tensor(
            out=ld[:], in0=ar_bc[0:1, 0:1], in1=ar_bc[0:1, 1:2], op=Alu.subtract
        )
        nc.vector.tensor_scalar(
            out=ld[:], in0=ld[:], scalar1=0.5, scalar2=None, op0=Alu.mult
        )
        nc.sync.dma_start(loss_o.ap().rearrange("(a k) -> a k", a=1), ld[:])

        a_t = singles.tile([P, 1], F32)
        nc.vector.tensor_scalar(
            out=a_t[:], in0=ar_bc[:, 0:1], scalar1=20.0, scalar2=None, op0=Alu.is_gt
        )
        b_t = singles.tile([P, 1], F32)
        nc.vector.tensor_scalar(
            out=b_t[:], in0=ar_bc[:, 1:2], scalar1=100.0, scalar2=None, op0=Alu.is_lt
        )
        negpi_t = singles.tile([P, 1], F32)
        nc.vector.memset(negpi_t[:], -PI)
        ssq_t = singles.tile([P, B], F32)
        nc.scalar.activation(
            out=ssq_t[:], in_=mnew_t[:], func=Act.Square, bias=negpi_t[:, 0:1],
            scale=PI,
        )
        rn_t = singles.tile([P, B], F32)
        nc.vector.tensor_scalar(
            out=rn_t[:], in0=ssq_t[:], scalar1=-PI, scalar2=-PI,
            op0=Alu.mult, op1=Alu.add,
        )
        sneg_t = singles.tile([P, B], F32)
        nc.vector.reciprocal(out=sneg_t[:], in_=rn_t[:])

        ga_t = singles.tile([P, B], F32)
        nc.vector.tensor_scalar(
            out=ga_t[:], in0=pms_t[:], scalar1=a_t[:, 0:1], scalar2=None, op0=Alu.mult
        )
        nms_t = singles.tile([P, B], F32)
        nc.vector.tensor_tensor(
            out=nms_t[:], in0=negt_t[:], in1=st_t[:], op=Alu.subtract
        )
        h_t = singles.tile([P, B], F32)
        nc.vector.scalar_tensor_tensor(
            out=h_t[:], in0=nms_t[:], scalar=b_t[:, 0:1], op0=Alu.mult,
            in1=ga_t[:], op1=Alu.subtract,
        )
        g_t = singles.tile([P, B], F32)
        nc.vector.tensor_tensor(
            out=g_t[:], in0=h_t[:], in1=sneg_t[:], op=Alu.mult
        )

        nc.vector.scalar_tensor_tensor(
            out=inp_bc[:], in0=inp_bc[:], scalar=0.1, op0=Alu.mult,
            in1=tr_bc[:], op1=Alu.add,
        )

        for b in range(B):
            for j in range(NKCH):
                dw_t = dwpool.tile([P, KCH], F32, tag="dw")
                nc.scalar.activation(
                    out=dw_t[:],
                    in_=inp_bc[:, j * KCH:(j + 1) * KCH],
                    func=Act.Copy,
                    bias=0.0,
                    scale=g_t[:, b:b + 1],
                )
                nc.sync.dma_start(dW_r[:, b, j * KCH:(j + 1) * KCH], dw_t[:])

    nc.compile()
    return nc


def get_nc(kind="fast"):
    key = "nc_" + kind
    if key not in _cache:
        _cache[key] = _build_nc_fast() if kind == "fast" else _build_nc_fallback()
    return _cache[key]


def decide_flags(inputs):
    """Resolve the two branch flags on the host when provably possible.
    Returns (a, b) floats or None if a bound is inconclusive."""
    pred = np.asarray(inputs["prediction"], np.float64)
    negt = np.asarray(inputs["negative_spk_trace"], np.float64)
    sptr = np.asarray(inputs["spk_trace"], np.float64)

    S = float(np.sum((negt - pred) ** 2))
    # guard against fp32-accumulation flips near the threshold
    eps_s = 1e-3 * S + 1e-2
    if S > 100.0 + eps_s:
        b = 0.0
    elif S < 100.0 - eps_s:
        b = 1.0
    else:
        return None

    d0 = (sptr - pred) ** 2
    d1 = (sptr + 0.1 - pred) ** 2
    lmin = float(np.sum(np.minimum(d0, d1)))
    lmax = float(np.sum(np.maximum(d0, d1)))
    eps_l = 1e-3 * max(lmin, lmax) + 1e-2
    if lmin > 20.0 + eps_l:
        a = 1.0
    elif lmax < 20.0 - eps_l:
        a = 0.0
    else:
        return None
    return a, b


def make_in_maps(inputs, flags=None):
    W = np.asarray(inputs["W"], dtype=np.float32)
    inp = np.ascontiguousarray(np.asarray(inputs["inp"], dtype=np.float32))
    mem = np.asarray(inputs["mem"], dtype=np.float32)
    sptr = np.asarray(inputs["spk_trace"], dtype=np.float32)
    inptr = np.ascontiguousarray(np.asarray(inputs["inp_trace"], dtype=np.float32))
    pred = np.asarray(inputs["prediction"], dtype=np.float32)
    negt = np.asarray(inputs["negative_spk_trace"], dtype=np.float32)
    in_maps = []
    for c in range(NCORES):
        s = slice(c * SH, (c + 1) * SH)
        m = {
            "W": np.ascontiguousarray(W[s]),
            "inp": inp,
            "mem": np.ascontiguousarray(mem[s]),
            "sptr": np.ascontiguousarray(sptr[s]),
            "inptr": inptr,
            "pred": np.ascontiguousarray(pred[s]),
            "negt": np.ascontiguousarray(negt[s]),
        }
        if flags is not None:
            m["flags"] = np.asarray(flags, dtype=np.float32)
        in_maps.append(m)
    return in_maps


def assemble_fast(results):
    spk = np.concatenate([np.asarray(results[c]["spk_o"]).reshape(SH) for c in range(NCORES)])
    st = np.concatenate([np.asarray(results[c]["st_o"]).reshape(SH) for c in range(NCORES)])
    dW = np.concatenate(
        [np.asarray(results[c]["dW_o"]).reshape(SH, NIN) for c in range(NCORES)], axis=0
    )
    L = np.float32(0.0)
    S = np.float32(0.0)
    for c in range(NCORES):
        ls = np.asarray(results[c]["lsum_o"]).reshape(2).astype(np.float32)
        L = np.float32(L + ls[0])
        S = np.float32(S + ls[1])
    loss = np.float32((L - S) / np.float32(2.0))
    return spk.astype(np.float32), st.astype(np.float32), loss, dW.astype(np.float32)


def assemble_fallback(results):
    spk = np.concatenate([np.asarray(results[c]["spk_o"]).reshape(SH) for c in range(NCORES)])
    st = np.concatenate([np.asarray(results[c]["st_o"]).reshape(SH) for c in range(NCORES)])
    dW = np.concatenate(
        [np.asarray(results[c]["dW_o"]).reshape(SH, NIN) for c in range(NCORES)], axis=0
    )
    loss = np.float32(np.asarray(results[0]["loss_o"]).reshape(-1)[0])
    return spk.astype(np.float32), st.astype(np.float32), loss, dW.astype(np.float32)


def run(inputs, trace=False, trace_cores=None):
    from concourse import bass_utils
    flags = decide_flags(inputs)
    if flags is not None:
        nc = get_nc("fast")
        res = bass_utils.run_bass_kernel_spmd(
            nc, make_in_maps(inputs, flags=flags), list(range(NCORES)),
            trace=trace, trace_cores=trace_cores,
        )
        return res, "fast"
    nc = get_nc("fallback")
    res = bass_utils.run_bass_kernel_spmd(
        nc, make_in_maps(inputs), list(range(NCORES)),
        trace=trace, trace_cores=trace_cores,
    )
    return res, "fallback"


def kernel(**inputs):
    res, kind = run(inputs, trace=False)
    if kind == "fast":
        return assemble_fast(res.results)
    return assemble_fallback(res.results)
